# revision 33
# baseline (speedup 1.0000x reference)
"""DisentangledSelfAttention (DeBERTa-style) Trainium2 Bass kernel.

Sharding: data-parallel over batch B=8 -> one batch element per NeuronCore.
Positional tensors are batch-independent and computed (replicated) per core.

Key algebraic structure exploited:
  rel[i, j] = j - i + 511 depends only on (j - i), and for S=384 only
  rel indices 128..894 (767 values) are ever used.  So:
    Kp_flat[p]  = rel_pos_emb[128 + p] @ Wpk          (p in [0, 767))
    Qp_flat[p]  = rel_pos_emb[128 + p] @ Wpq
    c2p[b,h,i,j] = q[b,i,h] . Kp_flat[j-i+383, h]
    p2c[b,h,i,j] = Qp_flat[j-i+383, h] . k[b,j,h]
  c2p comes from qp[i,p] = q[i].Kp_flat[p]: each 128-row i-tile computes a
  512-wide window of qp, bounces it to DRAM [384x512], and reads the score
  block back with row pitch 511 instead of 512 (a strided "skew" read that
  turns the per-row diagonal shift into a flat 2D access pattern).  p2c
  likewise from kq[j,p'] = k[j].Qp_rev[p'] (Qp rows reversed), read back
  transposed [j, i] with the same skew trick, PE-transposed into a bf16
  PSUM tile, and merged into the scores on the vector engines.

All matmul operands are stored bf16 (PE runs 1 cycle/row for bf16 at any
free size, and transposes with a bf16 identity run at 1 cycle/row too);
accumulation stays fp32 in PSUM, softmax statistics stay fp32.  The
softmax scale (dh**-0.5) and q_bias/v_bias are folded into the weights
and biases on the host before upload.  Elementwise work is spread over
the DVE, Activation, and Pool (gpsimd) engines to keep PE the only
near-critical engine.
"""

import itertools
import os
import sys

import numpy as np

B, S, D, H = 8, 384, 768, 12
DH = D // H          # 64
MAX_POS = 512
NP = 767             # number of used relative positions (128..894)
SCALE = DH ** -0.5

NIT = S // 128       # 3 i/j tiles
NDT = D // 128       # 6 d tiles
NPP = 768            # positional axis padded to even
NW = 512             # per-i-tile window of the positional axis (511 used)
# chunks of the positional axis (PSUM free dim <= 512 fp32)
PCHUNKS = [(0, 384), (384, 384)]

_CACHE = {}


def _import_concourse():
    try:
        import concourse.bass  # noqa: F401
    except ImportError:
        for p in ("/opt/trn_rl_repo", "/root/.axon_site/_ro/trn_rl_repo"):
            if os.path.isdir(p) and p not in sys.path:
                sys.path.insert(0, p)
        import concourse.bass  # noqa: F401


def _build():
    """Build + finalize the per-core Bass program (identical on all cores)."""
    _import_concourse()
    import concourse.bass as bass
    import concourse.bacc as bacc
    import concourse.mybir as mybir
    import concourse.tile as tile
    from concourse.bass import ts
    from concourse.masks import make_identity
    from concourse.tile import add_dep_helper

    f32 = mybir.dt.float32
    bf16 = mybir.dt.bfloat16
    sdt = bf16           # matmul operand storage dtype
    bdt = bf16           # bounce dtype
    ADD = mybir.AluOpType.add
    EXP = mybir.ActivationFunctionType.Exp

    nc = bacc.Bacc("TRN2", target_bir_lowering=False, debug=False)

    # ---------------- DRAM I/O ----------------
    xT = nc.dram_tensor("xT", [D, S], sdt, kind="ExternalInput")
    wq = nc.dram_tensor("wq", [D, D], sdt, kind="ExternalInput")
    wk = nc.dram_tensor("wk", [D, D], sdt, kind="ExternalInput")
    wv = nc.dram_tensor("wv", [D, D], sdt, kind="ExternalInput")
    wpk = nc.dram_tensor("wpk", [D, D], sdt, kind="ExternalInput")
    wpq = nc.dram_tensor("wpq", [D, D], sdt, kind="ExternalInput")
    wo = nc.dram_tensor("wo", [D, D], sdt, kind="ExternalInput")
    bq = nc.dram_tensor("bq", [D], f32, kind="ExternalInput")
    bk = nc.dram_tensor("bk", [D], f32, kind="ExternalInput")
    bv = nc.dram_tensor("bv", [D], f32, kind="ExternalInput")
    bo = nc.dram_tensor("bo", [D], f32, kind="ExternalInput")
    relkT = nc.dram_tensor("relkT", [D, NPP], sdt, kind="ExternalInput")
    out = nc.dram_tensor("out", [S, D], f32, kind="ExternalOutput")

    # per-head DRAM scratch for the skew bounce
    qp_dram = [nc.dram_tensor(f"qp_scratch_{h}", [S, NW], bdt) for h in range(H)]
    kq_dram = [nc.dram_tensor(f"kq_scratch_{h}", [S, NW], mybir.dt.float32r)
               for h in range(H)]

    with tile.TileContext(nc) as tc:
        with (
            tc.tile_pool(name="const", bufs=1) as constp,
            tc.tile_pool(name="big", bufs=1) as bigp,
            tc.tile_pool(name="wpool", bufs=2) as wpool,
            tc.tile_pool(name="bigtmp", bufs=3) as bigtmp,
            tc.tile_pool(name="work", bufs=3) as workp,
            tc.tile_pool(name="small", bufs=4) as smallp,
            tc.tile_pool(name="psA", bufs=2, space="PSUM") as psA,
            tc.tile_pool(name="psSC", bufs=2, space="PSUM") as psSC,
            tc.tile_pool(name="psB", bufs=3, space="PSUM") as psB,
            tc.tile_pool(name="psAV", bufs=1, space="PSUM") as psAV,
        ):
            import concourse.bass as bass_mod

            def psum(tag, shape=None, dtype=f32):
                pool = {"ps": psA, "sc": psSC, "ptwt": psB, "avps": psAV}[tag]
                return pool.tile(shape or [128, NW], dtype, tag=tag, name=tag)

            # round-robin PSUM->SBUF copy engines (DVE / Act / Pool)
            order = itertools.cycle(
                (nc.vector.tensor_copy, nc.scalar.copy)
            )

            def copy_rr(dst, src):
                next(order)(dst, src)

            qT_sb = bigp.tile([128, NDT, S], sdt, tag="qT")
            kT_sb = bigp.tile([128, NDT, S], sdt, tag="kT")
            v_sb = bigp.tile([128, NIT, D], sdt, tag="v")
            KpT_sb = bigp.tile([128, NDT, NPP], sdt, tag="KpT")
            QpTr_sb = bigp.tile([128, NDT, NPP], sdt, tag="QpTr")
            attnT_sb = bigp.tile([128, NDT, S], sdt, tag="attnT")

            # ---------- stage 1+2: projections & positional projections ----
            xT_sb = bigtmp.tile([128, NDT, S], sdt, tag="bigtmp")
            for ko in range(NDT):
                nc.sync.dma_start(xT_sb[:, ko, :], xT[ts(ko, 128), :])
            ident = constp.tile([128, 128], bf16, tag="ident")
            make_identity(nc, ident[:])
            ident32 = constp.tile([128, 128], f32, tag="ident32")
            make_identity(nc, ident32[:])
            identr = constp.tile([128, 128], mybir.dt.float32r, tag="identr")
            nc.vector.tensor_copy(identr[:], ident32[:])

            bq_sb = constp.tile([128, NDT], f32, tag="bq")
            bk_sb = constp.tile([128, NDT], f32, tag="bk")
            bvf = constp.tile([1, D], f32, tag="bv")
            bof = constp.tile([1, D], f32, tag="bo")
            bvr = constp.tile([128, D], f32, tag="bvr")
            bor = constp.tile([128, D], f32, tag="bor")
            nc.sync.dma_start(bq_sb[:], bq[:].rearrange("(o p) -> p o", p=128))
            nc.sync.dma_start(bk_sb[:], bk[:].rearrange("(o p) -> p o", p=128))
            nc.sync.dma_start(bvf[:], bv[:].unsqueeze(0))
            nc.sync.dma_start(bof[:], bo[:].unsqueeze(0))
            nc.gpsimd.partition_broadcast(bvr[:], bvf[:])
            nc.gpsimd.partition_broadcast(bor[:], bof[:])

            def qk_proj(wdram, bias_sb, dst):
                w_sb = wpool.tile([128, NDT, D], sdt, tag="w")
                for ko in range(NDT):
                    nc.sync.dma_start(w_sb[:, ko, :], wdram[ts(ko, 128), :])
                for mo in range(NDT):
                    ps_t = psum("ps")
                    for ko in range(NDT):
                        nc.tensor.matmul(
                            ps_t[:, :S],
                            w_sb[:, ko, ts(mo, 128)],
                            xT_sb[:, ko, :],
                            start=(ko == 0),
                            stop=(ko == NDT - 1),
                        )
                    nc.vector.tensor_scalar_add(
                        dst[:, mo, :], ps_t[:, :S], bias_sb[:, mo : mo + 1]
                    )

            def pos_proj(wdram, use_rel, dst):
                w_sb = wpool.tile([128, NDT, D], sdt, tag="w")
                for ko in range(NDT):
                    nc.sync.dma_start(w_sb[:, ko, :], wdram[ts(ko, 128), :])
                for mo in range(NDT):
                    for ci, (cs, csz) in enumerate(PCHUNKS):
                        ps_t = psum("ps")
                        for ko in range(NDT):
                            nc.tensor.matmul(
                                ps_t[:, :csz],
                                w_sb[:, ko, ts(mo, 128)],
                                use_rel[:, ko, cs : cs + csz],
                                start=(ko == 0),
                                stop=(ko == NDT - 1),
                            )
                        copy_rr(dst[:, mo, cs : cs + csz], ps_t[:, :csz])

            # ---------- stages 3-5: attention per head ---------------------
            qp_w = [None] * H
            kq_w = [None] * H
            skew_qp = {}     # h -> c2p_sb
            skew_kq = {}     # h -> p2ct_sb

            def head_slices(h):
                hp = 64 * (h % 2)
                ho = h // 2
                return hp, ho

            def stage3(pair, which):
                """qp (or kq) windowed matmuls + bounce to DRAM + skew-read
                issue for heads 2p, 2p+1.

                For i-tile t only positional columns [256-128t, 768-128t)
                are ever read back, so each row tile computes a 512-wide
                window and the bounce rows are stored with pitch 512.
                The skew reads queue behind the bounce writes so PE never
                waits on the DRAM round-trip later.
                """
                sdt_b = bdt if which == 0 else mybir.dt.float32r
                sbs = {}
                for sub in range(2):
                    sbs[sub] = workp.tile(
                        [128, NIT, NW], sdt_b,
                        tag=f"bounce{which}", name=f"bounce{which}",
                        bufs=5 if which == 0 else 4,
                    )
                for it in range(NIT):
                    w0 = 256 - 128 * it
                    for sub in range(2):
                        h = 2 * pair + sub
                        hp, ho = head_slices(h)
                        lhsT = (qT_sb if which == 0 else kT_sb)[
                            hp : hp + 64, ho, ts(it, 128)
                        ]
                        rhs = (KpT_sb if which == 0 else QpTr_sb)[
                            hp : hp + 64, ho, w0 : w0 + NW
                        ]
                        ps_t = psum("ps")
                        nc.tensor.matmul(
                            ps_t[:], lhsT, rhs, start=True, stop=True
                        )
                        copy_rr(sbs[sub][:, it, :], ps_t[:])
                # skew reads: c2p[t][ip, jf] / p2cT[t][u][jp, if]; flat addr
                # in [384, 512]: 127 + 511*row + 65536*tile + col
                for sub in range(2):
                    h = 2 * pair + sub
                    dram = (qp_dram if which == 0 else kq_dram)[h]
                    w_inst = nc.sync.dma_start(
                        dram[:].rearrange("(o p) c -> p o c", p=128),
                        sbs[sub][:],
                    )
                    rd = workp.tile(
                        [128, NIT, S], sdt_b,
                        tag="c2p" if which == 0 else "p2ct",
                        name="c2p" if which == 0 else "p2ct",
                        bufs=6 if which == 0 else 4,
                    )
                    r = nc.sync.dma_start(
                        rd[:],
                        bass_mod.AP(
                            dram, 127,
                            [[511, 128], [128 * NW, NIT], [1, S]],
                        ),
                    )
                    add_dep_helper(r.ins, w_inst.ins, reason="bounce raw")
                    if which == 0:
                        qp_w[h] = w_inst
                        skew_qp[h] = rd
                    else:
                        kq_w[h] = w_inst
                        skew_kq[h] = rd

            def A_t(ctx, t):
                """PE: c2c + p2c transpose-accumulate; merge TT; exp."""
                h, exps = ctx
                hp, ho = head_slices(h)
                c2p_sb = skew_qp[h]
                p2ct_sb = skew_kq[h]
                sc_ps = psum("sc", shape=[128, S])
                nc.tensor.matmul(
                    sc_ps[:, :S],
                    qT_sb[hp : hp + 64, ho, ts(t, 128)],
                    kT_sb[hp : hp + 64, ho, :],
                    start=True,
                    stop=False,
                    skip_group_check=True,
                )
                for u in range(NIT):
                    nc.tensor.matmul(
                        sc_ps[:, ts(u, 128)].bitcast(mybir.dt.float32r),
                        p2ct_sb[:, u, ts(t, 128)],
                        identr[:],
                        is_transpose=True,
                        start=False,
                        stop=(u == NIT - 1),
                        skip_group_check=True,
                    )
                s2 = workp.tile([128, S], f32, tag="s2", bufs=4)
                nc.vector.tensor_tensor(s2[:], sc_ps[:, :S], c2p_sb[:, t, :], ADD)
                exp_bf = workp.tile([128, S], bf16, tag="exp", bufs=12)
                ssum = smallp.tile([128, 1], f32, tag="ssum", bufs=10)
                nc.scalar.activation(exp_bf[:], s2[:], EXP, accum_out=ssum[:])
                exps.append((exp_bf, ssum))

            def N_t(ctx, t):
                """Normalize (recip + scale), two heads behind phaseA."""
                _h, exps = ctx
                exp_bf, ssum = exps[t]
                sinv = smallp.tile([128, 1], f32, tag="sinv", bufs=10)
                nc.vector.reciprocal(sinv[:], ssum[:])
                nc.gpsimd.tensor_scalar_mul(exp_bf[:], exp_bf[:], sinv[:])

            def B_t(ctx, t):
                """PE: transpose normalized weights, three heads behind."""
                h, exps, wT_sb = ctx
                exp_bf, _ssum = exps[t]
                wt_ps = psum("ptwt", shape=[128, S], dtype=bf16)
                for u in range(NIT):
                    nc.tensor.matmul(
                        wt_ps[:, ts(u, 128)],
                        exp_bf[:, ts(u, 128)],
                        ident[:],
                        is_transpose=True,
                        start=True,
                        stop=True,
                    )
                copy_rr(wT_sb[:, :, ts(t, 128)], wt_ps[:, :S])

            def B_av(ctx):
                """AV for a finished head -> attnT."""
                h, _exps, wT_sb = ctx
                hp, ho = head_slices(h)
                av_ps = psum("avps")
                for u in range(NIT):
                    nc.tensor.matmul(
                        av_ps[hp : hp + 64, :S],
                        v_sb[:, u, h * DH : (h + 1) * DH],
                        wT_sb[:, u, :],
                        start=(u == 0),
                        stop=(u == NIT - 1),
                    )
                if h % 2 == 0:
                    nc.vector.tensor_copy(
                        attnT_sb[hp : hp + 64, ho, :],
                        av_ps[hp : hp + 64, :S],
                    )
                else:
                    nc.scalar.copy(
                        attnT_sb[hp : hp + 64, ho, :],
                        av_ps[hp : hp + 64, :S],
                    )

            def step(ctxA, ctxN, ctxB):
                """One software-pipeline step, interleaved per t so PE fills
                the copy-drain windows of the lagging phases."""
                for t in range(NIT):
                    if ctxA is not None:
                        A_t(ctxA, t)
                    if ctxN is not None:
                        N_t(ctxN, t)
                    if ctxB is not None:
                        B_t(ctxB, t)
                if ctxB is not None:
                    B_av(ctxB)
                if ctxA is not None:
                    skew_qp.pop(ctxA[0])
                    skew_kq.pop(ctxA[0])

            # ---------- stage 1+2 execution ----------
            qk_proj(wq, bq_sb, qT_sb)
            qk_proj(wk, bk_sb, kT_sb)
            rel_sb = bigtmp.tile([128, NDT, NPP], sdt, tag="bigtmp",
                                 name="rel0")
            for ko in range(NDT):
                nc.sync.dma_start(rel_sb[:, ko, :], relkT[ts(ko, 128), :])
            pos_proj(wpk, rel_sb, KpT_sb)
            # reversed rel built on-chip: rel_rev[p'] = rel_fwd[766 - p'];
            # col 767 copied from the fwd pad
            rev_sb = bigtmp.tile([128, NDT, NPP], sdt, tag="bigtmp",
                                 name="rel1")
            nc.vector.tensor_copy(
                rev_sb[:, :, NPP - 1 : NPP], rel_sb[:, :, NPP - 1 : NPP]
            )
            for ko in range(NDT):
                fwd = rel_sb[:, ko, 0 : NPP - 1]
                rev = bass.AP(
                    fwd.tensor,
                    fwd.offset + (NPP - 2),
                    [[fwd.ap[0][0], 128], [-1, NPP - 1]],
                )
                nc.vector.tensor_copy(rev_sb[:, ko, 0 : NPP - 1], rev)
            pos_proj(wpq, rev_sb, QpTr_sb)

            # bounce the first pairs while v / wo work still fills PE+DMA
            for p in range(2):
                stage3(p, 1)
                stage3(p, 0)

            # v : [i(part), dout]  (bias along free dim; only needed from
            # the first phaseB on, so it runs after the first bounces)
            w_sb = wpool.tile([128, NDT, D], sdt, tag="w")
            for ko in range(NDT):
                nc.sync.dma_start(w_sb[:, ko, :], wv[ts(ko, 128), :])
            for io in range(NIT):
                for no in range(2):
                    ps_t = psum("ps")
                    for ko in range(NDT):
                        nc.tensor.matmul(
                            ps_t[:, :384],
                            xT_sb[:, ko, ts(io, 128)],
                            w_sb[:, ko, ts(no, 384)],
                            start=(ko == 0),
                            stop=(ko == NDT - 1),
                        )
                    nc.vector.tensor_tensor(
                        v_sb[:, io, ts(no, 384)],
                        ps_t[:, :384],
                        bvr[:, ts(no, 384)],
                        ADD,
                    )

            # prefetch Wo: fills remaining DMA lull instead of serializing
            # into the kernel tail
            wo_sb = wpool.tile([128, NDT, D], sdt, tag="w", name="wo_sb")
            for ko in range(NDT):
                nc.sync.dma_start(wo_sb[:, ko, :], wo[ts(ko, 128), :])
            stage3(2, 1)
            stage3(2, 0)

            # software pipeline: stage3 three pairs ahead; normalize two
            # heads and weight-transpose/AV three heads behind phaseA
            states = []
            NLAG, BLAG = 2, 3
            for i in range(H + BLAG):
                if i < H:
                    if i % 2 == 0:
                        pair = i // 2
                        if pair + 3 < 6:
                            stage3(pair + 3, 1)
                            stage3(pair + 3, 0)
                    ctxA = (i, [])
                    states.append(ctxA)
                else:
                    ctxA = None
                ctxN = None
                if 0 <= i - NLAG < H:
                    h, exps = states[i - NLAG]
                    ctxN = (h, exps)
                ctxB = None
                if 0 <= i - BLAG < H:
                    h, exps = states[i - BLAG]
                    wT_sb = workp.tile([128, NIT, S], sdt, tag="wT", bufs=4)
                    ctxB = (h, exps, wT_sb)
                step(ctxA, ctxN, ctxB)

            # ---------- stage 6: output projection --------------------
            w_sb = wo_sb
            for io in range(NIT):
                for no in range(2):
                    ps_t = psum("ps")
                    for ko in range(NDT):
                        nc.tensor.matmul(
                            ps_t[:, :384],
                            attnT_sb[:, ko, ts(io, 128)],
                            w_sb[:, ko, ts(no, 384)],
                            start=(ko == 0),
                            stop=(ko == NDT - 1),
                        )
                    o_sb = workp.tile([128, 384], f32, tag="osb")
                    nc.vector.tensor_tensor(
                        o_sb[:],
                        ps_t[:, :384],
                        bor[:, ts(no, 384)],
                        ADD,
                    )
                    nc.sync.dma_start(
                        out[ts(io, 128), ts(no, 384)], o_sb[:]
                    )

    nc.finalize()
    return nc


def _get_program(mm_dtype_name=None):
    key = "nc"
    if key not in _CACHE:
        _CACHE[key] = _build()
    return _CACHE[key]


def _host_prep(inputs):
    import ml_dtypes

    f = np.float32
    bf = ml_dtypes.bfloat16
    x = np.asarray(inputs["x"], f)
    rel = np.asarray(inputs["rel_pos_emb"], f)
    rel_used = rel[MAX_POS - S : MAX_POS - S + NP]          # rows 128..894
    base = {
        "wq": np.ascontiguousarray(np.asarray(inputs["Wq"], f) * SCALE).astype(bf),
        "wk": np.ascontiguousarray(np.asarray(inputs["Wk"], f)).astype(bf),
        "wv": np.ascontiguousarray(np.asarray(inputs["Wv"], f)).astype(bf),
        "wpk": np.ascontiguousarray(np.asarray(inputs["Wpk"], f)).astype(bf),
        "wpq": np.ascontiguousarray(np.asarray(inputs["Wpq"], f) * SCALE).astype(bf),
        "wo": np.ascontiguousarray(np.asarray(inputs["Wo"], f)).astype(bf),
        "bq": ((np.asarray(inputs["bq"], f) + np.asarray(inputs["q_bias"], f))
               * SCALE).astype(f),
        "bk": np.asarray(inputs["bk"], f),
        "bv": (np.asarray(inputs["bv"], f) + np.asarray(inputs["v_bias"], f)
               ).astype(f),
        "bo": np.asarray(inputs["bo"], f),
        "relkT": np.ascontiguousarray(
            np.pad(rel_used.T, ((0, 0), (0, 1)))).astype(bf),
    }
    in_maps = []
    for b in range(B):
        m = dict(base)
        m["xT"] = np.ascontiguousarray(x[b].T).astype(bf)
        in_maps.append(m)
    return in_maps


def _get_runner():
    """Build (once) a jitted SPMD executor for the compiled program.

    Mirrors concourse.bass2jax.run_bass_via_pjrt's multi-core path but caches
    the jitted callable so repeated kernel() calls don't re-trace/re-compile.
    """
    key = "runner"
    if key in _CACHE:
        return _CACHE[key]
    _import_concourse()
    import jax
    import jax.numpy as jnp  # noqa: F401
    from jax.sharding import Mesh, PartitionSpec
    from jax.experimental.shard_map import shard_map
    import concourse.mybir as mybir
    from concourse import bass2jax

    nc = _get_program()
    bass2jax.install_neuronx_cc_hook()

    partition_name = (
        nc.partition_id_tensor.name if nc.partition_id_tensor else None
    )
    in_names, out_names, out_avals, zero_outs = [], [], [], []
    for alloc in nc.m.functions[0].allocations:
        if not isinstance(alloc, mybir.MemoryLocationSet):
            continue
        name = alloc.memorylocations[0].name
        if alloc.kind == "ExternalInput":
            if name != partition_name:
                in_names.append(name)
        elif alloc.kind == "ExternalOutput":
            out_names.append(name)
            shape = tuple(alloc.tensor_shape)
            dtype = mybir.dt.np(alloc.dtype)
            out_avals.append(jax.core.ShapedArray(shape, dtype))
            zero_outs.append(np.zeros(shape, dtype))
    n_params = len(in_names)
    all_names = in_names + out_names
    if partition_name is not None:
        all_names = all_names + [partition_name]

    def _body(*args):
        operands = list(args)
        if partition_name is not None:
            operands.append(bass2jax.partition_id_tensor())
        outs = bass2jax._bass_exec_p.bind(
            *operands,
            out_avals=tuple(out_avals),
            in_names=tuple(all_names),
            out_names=tuple(out_names),
            lowering_input_output_aliases=(),
            sim_require_finite=True,
            sim_require_nnan=True,
            nc=nc,
        )
        return tuple(outs)

    devices = jax.devices()[:B]
    mesh = Mesh(np.asarray(devices), ("core",))
    n_outs = len(out_names)
    sharded = jax.jit(
        shard_map(
            _body,
            mesh=mesh,
            in_specs=(PartitionSpec("core"),) * (n_params + n_outs),
            out_specs=(PartitionSpec("core"),) * n_outs,
            check_rep=False,
        ),
        donate_argnums=tuple(range(n_params, n_params + n_outs)),
        keep_unused=True,
    )

    def run(in_maps):
        concat_in = [
            np.concatenate([np.asarray(in_maps[c][nm]) for c in range(B)], axis=0)
            for nm in in_names
        ]
        concat_zeros = [
            np.zeros((B * z.shape[0], *z.shape[1:]), z.dtype) for z in zero_outs
        ]
        out_arrs = sharded(*concat_in, *concat_zeros)
        return [
            {
                nm: np.asarray(out_arrs[i]).reshape(B, *out_avals[i].shape)[c]
                for i, nm in enumerate(out_names)
            }
            for c in range(B)
        ]

    _CACHE[key] = run
    return run


def _run(inputs, trace=False):
    run = _get_runner()
    in_maps = _host_prep(inputs)
    results = run(in_maps)
    outs = np.stack([np.asarray(results[b]["out"]) for b in range(B)])
    return outs.astype(np.float32), None


def kernel(**inputs) -> np.ndarray:
    out, _ = _run(inputs)
    return out
